# revision 1
# baseline (speedup 1.0000x reference)
"""BinaryTreeLSTMCell fused kernel for 8 TRN2 NeuronCores.

Strategy: data-parallel over the batch (8192 rows -> 1024 rows/core).
Per core, compute gates^T = W @ [x|h_left|h_right]^T (K=3072 contraction,
5120 gate rows) with fp32r matmuls (1 cycle/row at free>=256, ~tf32
precision), bias+sigmoid/tanh fused on ScalarE straight out of PSUM,
LSTM cell elementwise on VectorE, all in a gate-major (transposed)
layout so the contraction dim sits on SBUF partitions for both matmul
operands. Host pre-transposes the activations and pre-tiles W so every
DMA is wide and contiguous.
"""

import numpy as np

import concourse.bacc as bacc
import concourse.mybir as mybir
import concourse.tile as tile
from concourse.bass_utils import run_bass_kernel_spmd

F32 = mybir.dt.float32
F32R = mybir.dt.float32r
AF = mybir.ActivationFunctionType

N_CORES = 8
B = 8192
IN_SIZE = 1024
HID = 1024
COMB = IN_SIZE + 2 * HID          # 3072 contraction dim
NGATES = 5 * HID                  # 5120 stacked gate rows
BS = B // N_CORES                 # 1024 batch rows per core
KT = COMB // 128                  # 24 k-tiles
NT = NGATES // 128                # 40 gate tiles
JT = HID // 128                   # 8 h-slices
BB = BS // 512                    # 2 moving blocks of 512

_NC = {}


def _build(repeat=1):
    if repeat in _NC:
        return _NC[repeat]

    nc = bacc.Bacc("TRN2", target_bir_lowering=False, debug=False)

    combT = nc.dram_tensor("combT", [COMB, BS], F32R, kind="ExternalInput").ap()
    wbig = nc.dram_tensor("wbig", [NT, 128, COMB], F32R, kind="ExternalInput").ap()
    bias = nc.dram_tensor("bias", [128, NT], F32, kind="ExternalInput").ap()
    # c_left/c_right slices packed per h-slice j: one DMA loads both.
    ccT = nc.dram_tensor("ccT", [JT, 128, 2 * BS], F32, kind="ExternalInput").ap()
    # c (cols 0:BS) and h (cols BS:2BS) packed per h-slice: one DMA stores both.
    hcT = nc.dram_tensor("hcT", [JT, 128, 2 * BS], F32, kind="ExternalOutput").ap()

    with tile.TileContext(nc) as tc:
        with (
            tc.tile_pool(name="const", bufs=1) as const_pool,
            tc.tile_pool(name="comb", bufs=1) as comb_pool,
            tc.tile_pool(name="w", bufs=3) as w_pool,
            tc.tile_pool(name="gates", bufs=2) as gate_pool,
            tc.tile_pool(name="cc", bufs=2) as cc_pool,
            tc.tile_pool(name="ew", bufs=1) as ew_pool,
            tc.tile_pool(name="psum", bufs=8, space="PSUM") as psum_pool,
        ):
            bias_sb = const_pool.tile([128, NT], F32, tag="bias")
            nc.scalar.dma_start(bias_sb[:], bias[:])

            # Prefetch the first gates' weights ahead of the bulk comb load
            # so TensorE can start as soon as the first k-tiles land.
            wt_pre = {}
            for g in (4, 0):
                wt = w_pool.tile([128, COMB], F32R, tag="wt", name=f"wtpre{g}")
                nc.sync.dma_start(wt[:], wbig[g * JT])
                wt_pre[g] = wt

            # Load the first batch-half of every k-tile first so the first
            # accumulation groups (which read only columns [0,512)) can
            # start while the second half streams in. Separate tiles keep
            # the scheduler's DMA->matmul dependencies per-k-tile.
            comb_sb = []
            for k in range(KT):
                ct = comb_pool.tile([128, BS], F32R, tag=f"comb{k}")
                comb_sb.append(ct)
            for bb in range(BB):
                for k in range(KT):
                    nc.scalar.dma_start(
                        comb_sb[k][:, bb * 512:(bb + 1) * 512],
                        combT[k * 128:(k + 1) * 128, bb * 512:(bb + 1) * 512],
                    )

            def mm_group(wt, gt, g, n, bb):
                ps = psum_pool.tile([128, 512], F32, tag="ps", name=f"ps{n}_{bb}")
                for k in range(KT):
                    nc.tensor.matmul(
                        ps[:],
                        wt[:, k * 128:(k + 1) * 128],
                        comb_sb[k][:, bb * 512:(bb + 1) * 512],
                        start=(k == 0),
                        stop=(k == KT - 1),
                    )
                nc.scalar.activation(
                    gt[:, bb * 512:(bb + 1) * 512],
                    ps[:],
                    AF.Tanh if g == 4 else AF.Sigmoid,
                    bias=bias_sb[:, n:n + 1],
                )

            # Gate order u,i,fl,fr,o: the c-accumulation on VectorE then
            # overlaps the remaining gates' matmuls, so only o*tanh(c) +
            # the store trail the final matmul of each h-slice.
            def ew_after(g, gates, c_t, h_t, tmp, cl_t, cr_t, j):
                if g == 0:
                    nc.vector.tensor_mul(c_t, gates[0][:], gates[4][:])
                elif g == 1:
                    nc.vector.tensor_mul(tmp[:], gates[1][:], cl_t)
                    nc.vector.tensor_add(c_t, c_t, tmp[:])
                elif g == 2:
                    nc.vector.tensor_mul(tmp[:], gates[2][:], cr_t)
                    nc.vector.tensor_add(c_t, c_t, tmp[:])
                    nc.scalar.activation(h_t, c_t, AF.Tanh)
                elif g == 3:
                    nc.vector.tensor_mul(h_t, gates[3][:], h_t)
                    nc.sync.dma_start(hcT[j], hc_t[:])

            first = True
            for j in [jj for _ in range(repeat) for jj in range(JT)]:
                cc_t = cc_pool.tile([128, 2 * BS], F32, tag="cc")
                nc.scalar.dma_start(cc_t[:], ccT[j])
                cl_t = cc_t[:, 0:BS]
                cr_t = cc_t[:, BS:2 * BS]

                hc_t = ew_pool.tile([128, 2 * BS], F32, tag="hc")
                tmp = ew_pool.tile([128, BS], F32, tag="tmp")
                c_t = hc_t[:, 0:BS]
                h_t = hc_t[:, BS:2 * BS]
                gates = {}

                order = [(g, bb) for g in (4, 0, 1, 2, 3) for bb in range(BB)]

                wts = {}
                for g, bb in order:
                    n = g * JT + j
                    if g not in wts:
                        if first and g in wt_pre:
                            wts[g] = wt_pre[g]
                        else:
                            wt = w_pool.tile([128, COMB], F32R, tag="wt",
                                             name=f"wt{n}")
                            nc.sync.dma_start(wt[:], wbig[n])
                            wts[g] = wt
                        gates[g] = gate_pool.tile([128, BS], F32, tag=f"g{g}",
                                                  name=f"g{n}")
                    mm_group(wts[g], gates[g], g, n, bb)
                    if bb == BB - 1:
                        ew_after(g, gates, c_t, h_t, tmp, cl_t, cr_t, j)
                first = False

    nc.compile()
    _NC[repeat] = nc
    return nc


def make_in_maps(x, h_left, c_left, h_right, c_right, W, b):
    x, h_left, c_left, h_right, c_right, W, b = (
        np.asarray(a, dtype=np.float32)
        for a in (x, h_left, c_left, h_right, c_right, W, b)
    )
    comb = np.concatenate([x, h_left, h_right], axis=1)
    # wbig[n, p, k*128+m] = W[n*128+m, k*128+p]: per gate-tile n, a
    # (128 kpart, 24*128) block whose partition lines are contiguous.
    wbig = np.ascontiguousarray(
        W.reshape(NT, 128, KT, 128).transpose(0, 3, 2, 1).reshape(NT, 128, COMB)
    )
    bias_arr = np.ascontiguousarray(b.reshape(NT, 128).T)
    in_maps = []
    for i in range(N_CORES):
        sl = slice(i * BS, (i + 1) * BS)
        clT = c_left[sl].T.reshape(JT, 128, BS)
        crT = c_right[sl].T.reshape(JT, 128, BS)
        in_maps.append({
            "combT": np.ascontiguousarray(comb[sl].T),
            "wbig": wbig,
            "bias": bias_arr,
            "ccT": np.ascontiguousarray(np.concatenate([clT, crT], axis=2)),
        })
    return in_maps


def kernel(x, h_left, c_left, h_right, c_right, W, b):
    nc = _build()
    in_maps = make_in_maps(x, h_left, c_left, h_right, c_right, W, b)
    res = run_bass_kernel_spmd(nc, in_maps, list(range(N_CORES)))
    hs, cs = [], []
    for i in range(N_CORES):
        hc = res.results[i]["hcT"]  # (JT, 128, 2*BS)
        cs.append(hc[:, :, :BS].reshape(HID, BS).T)
        hs.append(hc[:, :, BS:].reshape(HID, BS).T)
    h = np.ascontiguousarray(np.concatenate(hs, axis=0))
    c = np.ascontiguousarray(np.concatenate(cs, axis=0))
    return h, c



# revision 2
# speedup vs baseline: 2.3474x; 2.3474x over previous
"""BinaryTreeLSTMCell fused kernel for 8 TRN2 NeuronCores.

Strategy: 2D sharding — 4-way data-parallel over the batch x 2-way
tensor-parallel over the hidden (output) dim, so each core computes a
(2048 batch, 512 h) block of the output with no cross-core traffic.
Per core, gates^T = W_half @ [x|h_left|h_right]^T runs in fp8-E4M3
with perf_mode=DoubleRow (2 weights per PE cell, K=256 per matmul,
~1.8x bf16 throughput). Weights are pre-scaled by 2^10 on host so
their U(-1/sqrt(3072), ..) range clears e4m3's min-normal (0.0156);
the 2^-10 dequant rides the activation's scale operand together with
the bias add. Loop order k2-outer / moving-block-inner amortizes each
256-column LDWEIGHTS over 4 moving blocks. Gate nonlinearities fused
on ScalarE out of PSUM, LSTM cell elementwise on VectorE in fp32.
"""

import numpy as np
import ml_dtypes

import concourse.bacc as bacc
import concourse.mybir as mybir
import concourse.tile as tile
from concourse.bass_utils import run_bass_kernel_spmd

F32 = mybir.dt.float32
F8 = mybir.dt.float8e4
E4NP = ml_dtypes.float8_e4m3
AF = mybir.ActivationFunctionType
DR = mybir.MatmulPerfMode.DoubleRow

N_CORES = 8
B = 8192
IN_SIZE = 1024
HID = 1024
COMB = IN_SIZE + 2 * HID          # 3072 contraction dim
DP = 4                            # batch shards
TP = 2                            # hidden shards
BS = B // DP                      # 2048 batch rows per core
HS = HID // TP                    # 512 h-cols per core
KT = COMB // 128                  # 24 k-tiles
K2T = KT // 2                     # 12 double-k-tiles (256 each)
NT = 5 * HS // 128                # 20 gate tiles per core
JT = HS // 128                    # 4 h-subtiles per core
BBT = BS // 512                   # 4 moving blocks of 512
SW = 2.0 ** -10                   # weight dequant scale

_NC = {}


def _build(repeat=1):
    if repeat in _NC:
        return _NC[repeat]

    nc = bacc.Bacc("TRN2", target_bir_lowering=False, debug=False)

    # comb8[k2] = [128 kpart, 2 kplanes, BS batch] fp8 per double-k-tile.
    comb8 = nc.dram_tensor("comb8", [K2T, 128, 2, BS], F8, kind="ExternalInput").ap()
    # wbig8[n] = [128 kpart, 24 ktile, 128 m] fp8 per gate tile.
    wbig8 = nc.dram_tensor("wbig8", [NT, 128, KT, 128], F8, kind="ExternalInput").ap()
    bias = nc.dram_tensor("bias", [128, NT], F32, kind="ExternalInput").ap()
    # c_left/c_right slices packed per h-subtile j: one DMA loads both.
    ccT = nc.dram_tensor("ccT", [JT, 128, 2 * BS], F32, kind="ExternalInput").ap()
    # c (cols 0:BS) and h (cols BS:2BS) packed per h-subtile.
    hcT = nc.dram_tensor("hcT", [JT, 128, 2 * BS], F32, kind="ExternalOutput").ap()

    with tile.TileContext(nc) as tc:
        with (
            tc.tile_pool(name="const", bufs=1) as const_pool,
            tc.tile_pool(name="comb", bufs=1) as comb_pool,
            tc.tile_pool(name="w", bufs=3) as w_pool,
            tc.tile_pool(name="gates", bufs=2) as gate_pool,
            tc.tile_pool(name="cc", bufs=2) as cc_pool,
            tc.tile_pool(name="ew", bufs=1) as ew_pool,
            tc.tile_pool(name="psum", bufs=8, space="PSUM") as psum_pool,
        ):
            bias_sb = const_pool.tile([128, NT], F32, tag="bias")
            nc.scalar.dma_start(bias_sb[:], bias[:])

            # Prefetch the first gates' weights ahead of the bulk comb load
            # so TensorE can start as soon as the first k-tiles land.
            wt_pre = {}
            for g in (4, 0):
                wt = w_pool.tile([128, KT, 128], F8, tag="wt", name=f"wtpre{g}")
                nc.sync.dma_start(wt[:], wbig8[g * JT])
                wt_pre[g] = wt

            # Persistent comb tiles, one per double-k-tile so matmul deps are
            # per-k2. Split the loads across both DMA queues.
            comb_sb = []
            for k2 in range(K2T):
                ct = comb_pool.tile([128, 2, BS], F8, tag=f"comb{k2}")
                comb_sb.append(ct)
            for k2 in range(K2T):
                eng = nc.scalar if k2 % 2 == 0 else nc.sync
                eng.dma_start(comb_sb[k2][:], comb8[k2])

            def mm_gate(wt, gt, g, n):
                ps = [
                    psum_pool.tile([128, 512], F32, tag="ps", name=f"ps{n}_{bb}")
                    for bb in range(BBT)
                ]
                for k2 in range(K2T):
                    ws = wt[:, 2 * k2:2 * k2 + 2, :]
                    for bb in range(BBT):
                        nc.tensor.matmul(
                            ps[bb][:],
                            ws,
                            comb_sb[k2][:, :, bb * 512:(bb + 1) * 512],
                            start=(k2 == 0),
                            stop=(k2 == K2T - 1),
                            perf_mode=DR,
                        )
                for bb in range(BBT):
                    nc.scalar.activation(
                        gt[:, bb * 512:(bb + 1) * 512],
                        ps[bb][:],
                        AF.Tanh if g == 4 else AF.Sigmoid,
                        bias=bias_sb[:, n:n + 1],
                        scale=SW,
                    )

            # Gate order u,i,fl,fr,o: the c-accumulation on VectorE then
            # overlaps the remaining gates' matmuls, so only o*tanh(c) +
            # the store trail the final matmul of each h-subtile.
            def ew_after(g, gates, c_t, h_t, tmp, cl_t, cr_t, hc_t, j):
                if g == 0:
                    nc.vector.tensor_mul(c_t, gates[0][:], gates[4][:])
                elif g == 1:
                    nc.vector.tensor_mul(tmp[:], gates[1][:], cl_t)
                    nc.vector.tensor_add(c_t, c_t, tmp[:])
                elif g == 2:
                    nc.vector.tensor_mul(tmp[:], gates[2][:], cr_t)
                    nc.vector.tensor_add(c_t, c_t, tmp[:])
                    nc.scalar.activation(h_t, c_t, AF.Tanh)
                elif g == 3:
                    nc.vector.tensor_mul(h_t, gates[3][:], h_t)
                    nc.sync.dma_start(hcT[j], hc_t[:])

            first = True
            for j in [jj for _ in range(repeat) for jj in range(JT)]:
                cc_t = cc_pool.tile([128, 2 * BS], F32, tag="cc")
                nc.scalar.dma_start(cc_t[:], ccT[j])
                cl_t = cc_t[:, 0:BS]
                cr_t = cc_t[:, BS:2 * BS]

                hc_t = ew_pool.tile([128, 2 * BS], F32, tag="hc")
                tmp = ew_pool.tile([128, BS], F32, tag="tmp")
                c_t = hc_t[:, 0:BS]
                h_t = hc_t[:, BS:2 * BS]
                gates = {}

                for g in (4, 0, 1, 2, 3):
                    n = g * JT + j
                    if first and g in wt_pre:
                        wt = wt_pre[g]
                    else:
                        wt = w_pool.tile([128, KT, 128], F8, tag="wt",
                                         name=f"wt{n}")
                        nc.sync.dma_start(wt[:], wbig8[n])
                    gt = gate_pool.tile([128, BS], F32, tag=f"g{g}",
                                        name=f"g{n}")
                    gates[g] = gt
                    mm_gate(wt, gt, g, n)
                    ew_after(g, gates, c_t, h_t, tmp, cl_t, cr_t, hc_t, j)
                first = False

    nc.compile()
    _NC[repeat] = nc
    return nc


def make_in_maps(x, h_left, c_left, h_right, c_right, W, b):
    x, h_left, c_left, h_right, c_right, W, b = (
        np.asarray(a, dtype=np.float32)
        for a in (x, h_left, c_left, h_right, c_right, W, b)
    )
    comb = np.concatenate([x, h_left, h_right], axis=1)
    comb8 = comb.astype(E4NP)

    # Per TP half t: gate-tile n = g*JT + j covers W rows
    # g*HID + t*HS + j*128 + m. wbig8[n, p, k, m] = Wq[n, m, k*128 + p].
    Wq = (W * (1.0 / SW)).astype(E4NP)
    Wq = Wq.reshape(5, TP, JT * 128, COMB)
    b5 = b.reshape(5, TP, JT, 128)

    in_maps = []
    for i in range(N_CORES):
        dp, t = divmod(i, TP)
        bsl = slice(dp * BS, (dp + 1) * BS)
        # comb8c[k2, p, two, b] = comb[bsl][b, (2*k2+two)*128 + p]
        comb8c = np.ascontiguousarray(
            comb8[bsl].T.reshape(K2T, 2, 128, BS).transpose(0, 2, 1, 3)
        )
        wcore = np.ascontiguousarray(
            Wq[:, t].reshape(NT, 128, KT, 128).transpose(0, 3, 2, 1)
        )
        bias_arr = np.ascontiguousarray(b5[:, t].reshape(NT, 128).T)
        hsl = slice(t * HS, (t + 1) * HS)
        clT = c_left[bsl, hsl].T.reshape(JT, 128, BS)
        crT = c_right[bsl, hsl].T.reshape(JT, 128, BS)
        in_maps.append({
            "comb8": comb8c,
            "wbig8": wcore,
            "bias": bias_arr,
            "ccT": np.ascontiguousarray(np.concatenate([clT, crT], axis=2)),
        })
    return in_maps


def kernel(x, h_left, c_left, h_right, c_right, W, b):
    nc = _build()
    in_maps = make_in_maps(x, h_left, c_left, h_right, c_right, W, b)
    res = run_bass_kernel_spmd(nc, in_maps, list(range(N_CORES)))
    h = np.empty((B, HID), np.float32)
    c = np.empty((B, HID), np.float32)
    for i in range(N_CORES):
        dp, t = divmod(i, TP)
        bsl = slice(dp * BS, (dp + 1) * BS)
        hsl = slice(t * HS, (t + 1) * HS)
        hc = res.results[i]["hcT"]  # (JT, 128, 2*BS)
        c[bsl, hsl] = hc[:, :, :BS].transpose(2, 0, 1).reshape(BS, HS)
        h[bsl, hsl] = hc[:, :, BS:].transpose(2, 0, 1).reshape(BS, HS)
    return h, c


# revision 3
# speedup vs baseline: 2.4388x; 1.0389x over previous
"""BinaryTreeLSTMCell fused kernel for 8 TRN2 NeuronCores.

Strategy: 2D sharding — 4-way data-parallel over the batch x 2-way
tensor-parallel over the hidden (output) dim, so each core computes a
(2048 batch, 512 h) block of the output with no cross-core traffic.
Per core, gates^T = W_half @ [x|h_left|h_right]^T runs in fp8-E4M3
with perf_mode=DoubleRow (2 weights per PE cell, K=256 per matmul,
~1.8x bf16 throughput). Weights are pre-scaled by 2^10 on host so
their U(-1/sqrt(3072), ..) range clears e4m3's min-normal (0.0156);
the 2^-10 dequant rides the activation's scale operand together with
the bias add. Loop order k2-outer / moving-block-inner amortizes each
256-column LDWEIGHTS over 4 moving blocks. Gate nonlinearities fused
on ScalarE out of PSUM, LSTM cell elementwise on VectorE in fp32.
"""

import numpy as np
import ml_dtypes

import concourse.bacc as bacc
import concourse.mybir as mybir
import concourse.tile as tile
from concourse.bass_utils import run_bass_kernel_spmd

F32 = mybir.dt.float32
F8 = mybir.dt.float8e4
E4NP = ml_dtypes.float8_e4m3
AF = mybir.ActivationFunctionType
DR = mybir.MatmulPerfMode.DoubleRow

N_CORES = 8
B = 8192
IN_SIZE = 1024
HID = 1024
COMB = IN_SIZE + 2 * HID          # 3072 contraction dim
DP = 4                            # batch shards
TP = 2                            # hidden shards
BS = B // DP                      # 2048 batch rows per core
HS = HID // TP                    # 512 h-cols per core
KT = COMB // 128                  # 24 k-tiles
K2T = KT // 2                     # 12 double-k-tiles (256 each)
NT = 5 * HS // 128                # 20 gate tiles per core
JT = HS // 128                    # 4 h-subtiles per core
BBT = BS // 512                   # 4 moving blocks of 512
SW = 2.0 ** -10                   # weight dequant scale

_NC = {}


def _build(repeat=1):
    if repeat in _NC:
        return _NC[repeat]

    nc = bacc.Bacc("TRN2", target_bir_lowering=False, debug=False)

    # comb8[k2] = [128 kpart, 2 kplanes, BS batch] fp8 per double-k-tile.
    comb8 = nc.dram_tensor("comb8", [K2T, 128, 2, BS], F8, kind="ExternalInput").ap()
    # wbig8[n] = [128 kpart, 24 ktile, 128 m] fp8 per gate tile.
    wbig8 = nc.dram_tensor("wbig8", [NT, 128, KT, 128], F8, kind="ExternalInput").ap()
    bias = nc.dram_tensor("bias", [128, NT], F32, kind="ExternalInput").ap()
    # c_left/c_right slices packed per h-subtile j: one DMA loads both.
    ccT = nc.dram_tensor("ccT", [JT, 128, 2 * BS], F32, kind="ExternalInput").ap()
    # c (cols 0:BS) and h (cols BS:2BS) packed per h-subtile.
    hcT = nc.dram_tensor("hcT", [JT, 128, 2 * BS], F32, kind="ExternalOutput").ap()

    with tile.TileContext(nc) as tc:
        with (
            tc.tile_pool(name="const", bufs=1) as const_pool,
            tc.tile_pool(name="comb", bufs=1) as comb_pool,
            tc.tile_pool(name="w", bufs=3) as w_pool,
            tc.tile_pool(name="gates", bufs=2) as gate_pool,
            tc.tile_pool(name="cc", bufs=2) as cc_pool,
            tc.tile_pool(name="ew", bufs=1) as ew_pool,
            tc.tile_pool(name="psum", bufs=8, space="PSUM") as psum_pool,
        ):
            bias_sb = const_pool.tile([128, NT], F32, tag="bias")
            nc.scalar.dma_start(bias_sb[:], bias[:])

            # Prefetch the first gates' weights ahead of the bulk comb load
            # so TensorE can start as soon as the first k-tiles land.
            wt_pre = {}
            for g in (4, 0):
                wt = w_pool.tile([128, KT, 128], F8, tag="wt", name=f"wtpre{g}")
                nc.sync.dma_start(wt[:], wbig8[g * JT])
                wt_pre[g] = wt

            # Persistent comb tiles, one per double-k-tile so matmul deps are
            # per-k2. Split the loads across both DMA queues.
            comb_sb = []
            for k2 in range(K2T):
                ct = comb_pool.tile([128, 2, BS], F8, tag=f"comb{k2}")
                comb_sb.append(ct)
            for k2 in range(K2T):
                eng = nc.scalar if k2 % 2 == 0 else nc.sync
                eng.dma_start(comb_sb[k2][:], comb8[k2])

            def mm_gate(wt, gt, g, n):
                ps = [
                    psum_pool.tile([128, 512], F32, tag="ps", name=f"ps{n}_{bb}")
                    for bb in range(BBT)
                ]
                for k2 in range(K2T):
                    ws = wt[:, 2 * k2:2 * k2 + 2, :]
                    for bb in range(BBT):
                        nc.tensor.matmul(
                            ps[bb][:],
                            ws,
                            comb_sb[k2][:, :, bb * 512:(bb + 1) * 512],
                            start=(k2 == 0),
                            stop=(k2 == K2T - 1),
                            perf_mode=DR,
                        )
                for bb in range(BBT):
                    nc.scalar.activation(
                        gt[:, bb * 512:(bb + 1) * 512],
                        ps[bb][:],
                        AF.Tanh if g == 4 else AF.Sigmoid,
                        bias=bias_sb[:, n:n + 1],
                        scale=SW,
                    )

            # Gate order u,i,fl,fr,o: the c-accumulation on VectorE then
            # overlaps the remaining gates' matmuls. c is stored as soon as
            # it is final (after fr), and the o*tanh(c) product + h store
            # are chunked so only one 512-col chunk trails the last matmul.
            def ew_after(g, gates, c_t, h_t, tmp, cl_t, cr_t, hc_t, j):
                if g == 0:
                    nc.vector.tensor_mul(c_t, gates[0][:], gates[4][:])
                elif g == 1:
                    nc.vector.tensor_mul(tmp[:], gates[1][:], cl_t)
                    nc.vector.tensor_add(c_t, c_t, tmp[:])
                elif g == 2:
                    nc.vector.tensor_mul(tmp[:], gates[2][:], cr_t)
                    nc.vector.tensor_add(c_t, c_t, tmp[:])
                    nc.scalar.activation(h_t, c_t, AF.Tanh)
                    nc.sync.dma_start(hcT[j][:, 0:BS], c_t)
                elif g == 3:
                    for q in range(BBT):
                        qs = slice(q * 512, (q + 1) * 512)
                        nc.vector.tensor_mul(h_t[:, qs], gates[3][:, qs],
                                             h_t[:, qs])
                        nc.sync.dma_start(hcT[j][:, BS + q * 512:BS + (q + 1) * 512],
                                          h_t[:, qs])

            first = True
            for j in [jj for _ in range(repeat) for jj in range(JT)]:
                cc_t = cc_pool.tile([128, 2 * BS], F32, tag="cc")
                nc.scalar.dma_start(cc_t[:], ccT[j])
                cl_t = cc_t[:, 0:BS]
                cr_t = cc_t[:, BS:2 * BS]

                hc_t = ew_pool.tile([128, 2 * BS], F32, tag="hc")
                tmp = ew_pool.tile([128, BS], F32, tag="tmp")
                c_t = hc_t[:, 0:BS]
                h_t = hc_t[:, BS:2 * BS]
                gates = {}

                for g in (4, 0, 1, 2, 3):
                    n = g * JT + j
                    if first and g in wt_pre:
                        wt = wt_pre[g]
                    else:
                        wt = w_pool.tile([128, KT, 128], F8, tag="wt",
                                         name=f"wt{n}")
                        nc.sync.dma_start(wt[:], wbig8[n])
                    gt = gate_pool.tile([128, BS], F32, tag=f"g{g}",
                                        name=f"g{n}")
                    gates[g] = gt
                    mm_gate(wt, gt, g, n)
                    ew_after(g, gates, c_t, h_t, tmp, cl_t, cr_t, hc_t, j)
                first = False

    nc.compile()
    _NC[repeat] = nc
    return nc


def make_in_maps(x, h_left, c_left, h_right, c_right, W, b):
    x, h_left, c_left, h_right, c_right, W, b = (
        np.asarray(a, dtype=np.float32)
        for a in (x, h_left, c_left, h_right, c_right, W, b)
    )
    comb = np.concatenate([x, h_left, h_right], axis=1)
    comb8 = comb.astype(E4NP)

    # Per TP half t: gate-tile n = g*JT + j covers W rows
    # g*HID + t*HS + j*128 + m. wbig8[n, p, k, m] = Wq[n, m, k*128 + p].
    Wq = (W * (1.0 / SW)).astype(E4NP)
    Wq = Wq.reshape(5, TP, JT * 128, COMB)
    b5 = b.reshape(5, TP, JT, 128)

    in_maps = []
    for i in range(N_CORES):
        dp, t = divmod(i, TP)
        bsl = slice(dp * BS, (dp + 1) * BS)
        # comb8c[k2, p, two, b] = comb[bsl][b, (2*k2+two)*128 + p]
        comb8c = np.ascontiguousarray(
            comb8[bsl].T.reshape(K2T, 2, 128, BS).transpose(0, 2, 1, 3)
        )
        wcore = np.ascontiguousarray(
            Wq[:, t].reshape(NT, 128, KT, 128).transpose(0, 3, 2, 1)
        )
        bias_arr = np.ascontiguousarray(b5[:, t].reshape(NT, 128).T)
        hsl = slice(t * HS, (t + 1) * HS)
        clT = c_left[bsl, hsl].T.reshape(JT, 128, BS)
        crT = c_right[bsl, hsl].T.reshape(JT, 128, BS)
        in_maps.append({
            "comb8": comb8c,
            "wbig8": wcore,
            "bias": bias_arr,
            "ccT": np.ascontiguousarray(np.concatenate([clT, crT], axis=2)),
        })
    return in_maps


def kernel(x, h_left, c_left, h_right, c_right, W, b):
    nc = _build()
    in_maps = make_in_maps(x, h_left, c_left, h_right, c_right, W, b)
    res = run_bass_kernel_spmd(nc, in_maps, list(range(N_CORES)))
    h = np.empty((B, HID), np.float32)
    c = np.empty((B, HID), np.float32)
    for i in range(N_CORES):
        dp, t = divmod(i, TP)
        bsl = slice(dp * BS, (dp + 1) * BS)
        hsl = slice(t * HS, (t + 1) * HS)
        hc = res.results[i]["hcT"]  # (JT, 128, 2*BS)
        c[bsl, hsl] = hc[:, :, :BS].transpose(2, 0, 1).reshape(BS, HS)
        h[bsl, hsl] = hc[:, :, BS:].transpose(2, 0, 1).reshape(BS, HS)
    return h, c
